# revision 27
# baseline (speedup 1.0000x reference)
"""Additive (Bahdanau) attention on 8 Trainium2 NeuronCores.

Strategy
--------
scores[b,q,k] = sum_h wv[h] * tanh(qp[b,q,h] + kp[b,k,h]) is evaluated via a
separable sin-basis expansion  tanh(S*x) ~ sum_m c_m sin(theta_m * x), so that
sin(w*(qp+kp)) = sin(w qp)cos(w kp) + cos(w qp)sin(w kp) turns the O(nq*nk*dh)
tanh work into PE matmuls over 2*M*dh fused feature columns.  Feature angles
are range-reduced to the hardware Sin table domain [-pi, pi] with an int32
fixed-point fraction trick on the vector engine.  Masked softmax is exact:
only valid key tiles are ever computed (work is bin-packed over cores at
128-key-tile granularity), the denominator comes from an extra all-ones
column in the PV matmul, and per-(core,slot) partial numerators/denominators
are summed and normalized on the host.

Each core runs the same program; per-core work differences are expressed
purely through host-staged input tensors.
"""

import os
import threading

import numpy as np

import concourse.bacc as bacc
import concourse.mybir as mybir
from concourse.tile import TileContext
from concourse.bass_utils import run_bass_kernel_spmd

# Problem constants (nn_AdditiveAttention_48859547959476).
B, NQ, NK, DQ, DK, DV, DH = 16, 128, 2048, 256, 256, 128, 64
N_CORES = 8
M = 8                  # sin modes per h
NPAIR = M // 2         # two modes packed per 128-partition instruction
FRAC_BITS = 18
QSC = float(2 ** FRAC_BITS)
TILE = 128             # key-tile granularity
UPS = 4                # units (key tiles) per slot; one slot = one batch segment

# Fitted sin-basis tables: tanh(S*x) ~ sum_m c_m * sin(theta_m * x), x in [-1,1].
# Data-independent constants (depend only on the interval half-width S grid).
THETA_TABLE = {
    6.5: ([3.17193937, 9.60585883, 2.18738104, 3.05597649, 14.9417154, 20.4502654, 26.1547075, 31.9750537],
           [36.2232438, 0.161350491, 6.87426557, -41.2400577, 0.0450386597, 0.0123209328, 0.0032060403, 0.000771167043]),
    7.0: ([6.72542412, 11.4488434, 0.860825183, 2.30642042, 16.467999, 21.7569349, 27.2808087, 32.9082271],
           [0.304060007, 0.107251691, 0.22853436, 1.09560141, 0.0366555054, 0.0117308915, 0.00351331224, 0.000959944543]),
    7.5: ([7.25768278, 12.6007308, 0.94197816, 0.389083886, 18.0598494, 23.6786715, 29.4473074, 35.2574239],
           [0.327218102, 0.103098701, 18.1001284, -36.5762453, 0.0336387636, 0.0106648096, 0.00325694801, 0.000927483766]),
    8.0: ([7.62827148, 12.9275591, 2.38525745, 0.0910723624, 18.386375, 24.0077522, 29.7720771, 35.5527541],
           [0.307919105, 0.106834021, 1.40006375, -2.92600933, 0.03750095, 0.0127778322, 0.00420384838, 0.00129178369]),
    8.5: ([8.57452428, 13.6394417, 3.96189912, 0.0316319922, 18.9814717, 24.5275185, 30.2180244, 35.8879972],
           [0.245844862, 0.0994266193, 0.631129752, 37.8639255, 0.0384797017, 0.0142488136, 0.00507491304, 0.00166977822]),
    9.0: ([10.5059062, 15.9257514, 5.22232869, 0.281150478, 21.4885597, 27.1845335, 32.985403, 38.7556262],
           [0.194960676, 0.0760384553, 0.556324013, 5.93191099, 0.0294364649, 0.0111261319, 0.00409146713, 0.00140742744]),
    9.5: ([10.6455037, 16.092124, 5.34363126, 0.631205216, 21.6748922, 27.381036, 33.1809849, 38.9279582],
           [0.200629478, 0.0815646864, 0.547398048, 2.77132144, 0.0330422178, 0.0131083071, 0.00507118009, 0.00183458507]),
    10.0: ([10.7886067, 16.2530977, 5.48579349, 0.913103266, 21.850584, 27.5632467, 33.3571549, 39.0815875],
           [0.204992033, 0.0866742822, 0.533127269, 2.02948997, 0.036611695, 0.0151849651, 0.00615193949, 0.00232647664]),
    10.5: ([10.9868662, 16.4488031, 5.73180365, 1.30190719, 22.0509821, 27.7667963, 33.5572858, 39.2669982],
           [0.205719804, 0.0906935634, 0.500097882, 1.5731762, 0.0398856941, 0.0172431936, 0.00729298612, 0.0028792393]),
}
SGRID = np.array(sorted(THETA_TABLE.keys()))

_prog_cache = {}
_prog_lock = threading.Lock()


def _build_program(nslot):
    """One Bass/Tile program shared by all 8 cores.

    Inputs (per core, staged by the host):
      kT_st   [2, 128, U*128] f16 : transposed keys (dk-tile major), zero-padded
      qT_st   [2, 128, S*128] f16 : transposed queries per slot
      vals_st [128, U*132]    f16 : per unit: 128 value cols + mask col + 3 pad
      wqk_st  [128, 1024]     f16 : [WqWq(2x128) | WkWk(2x128)] dup'd, 1/(2pi*Sg) folded
      aux     [128, A]        f32 : theta*2^18 pair vecs, U coefs, biases
    Output:
      out_part [128, S*132] f32 : per slot: 128 PV cols + denominator col + 3 pad
    """
    units = nslot * UPS
    KW = units * TILE          # staged key columns
    QW = nslot * TILE          # staged query columns
    AW = NPAIR + NPAIR + 2     # aux cols: theta pair vecs, ucoef pair vecs, biases
    f32, f16, i32 = mybir.dt.float32, mybir.dt.float16, mybir.dt.int32

    nc = bacc.Bacc("TRN2", target_bir_lowering=False, debug=False,
                   num_devices=N_CORES)
    kT_st = nc.dram_tensor("kT_st", [2, 128, KW], f16, kind="ExternalInput").ap()
    uq_st = nc.dram_tensor("uq_st", [128, 2 * NPAIR * QW], f16, kind="ExternalInput").ap()
    vals_st = nc.dram_tensor("vals_st", [128, units * 132], f16, kind="ExternalInput").ap()
    wqk_st = nc.dram_tensor("wqk_st", [128, 256], f16, kind="ExternalInput").ap()
    aux = nc.dram_tensor("aux", [128, AW], f32, kind="ExternalInput").ap()
    out_part = nc.dram_tensor("out_part", [128, nslot * 132], f32, kind="ExternalOutput").ap()

    NCH = KW // 512            # 512-col key-projection chunks
    FW = KW + QW               # fused feature width (keys then queries)

    with TileContext(nc) as tc:
        with (
            tc.tile_pool(name="const_sb", bufs=1) as csb,
            tc.tile_pool(name="big_sb", bufs=1) as bsb,
            tc.tile_pool(name="work_sb", bufs=2) as wsb,
            tc.tile_pool(name="kps_ps", bufs=3, space="PSUM") as kps_pool,
            tc.tile_pool(name="sc_ps", bufs=2, space="PSUM") as sc_pool,
            tc.tile_pool(name="pv_ps", bufs=1, space="PSUM") as pv_pool,
        ):
            auxt = csb.tile([128, AW], f32, name="auxt")
            nc.sync.dma_start(out=auxt, in_=aux[:, :])
            wqkt = csb.tile([128, 256], f16, name="wqkt")
            nc.sync.dma_start(out=wqkt, in_=wqk_st[:, :])
            kT_sb = bsb.tile([128, 2 * KW], f16, name="kT_sb")
            for c in range(NCH):
                for t in range(2):
                    nc.sync.dma_start(
                        out=kT_sb[:, t * KW + c * 512: t * KW + (c + 1) * 512],
                        in_=kT_st[t, :, c * 512:(c + 1) * 512])
            uq_sb = csb.tile([128, 2 * NPAIR * QW], f16, name="uq_sb")
            nc.sync.dma_start(out=uq_sb, in_=uq_st[:, :])
            vals_sb = bsb.tile([128, units * 132], f16, name="vals_sb")
            nc.sync.dma_start(out=vals_sb, in_=vals_st[:, :])

            th_ap = [auxt[:, p:p + 1] for p in range(NPAIR)]
            uc_ap = [auxt[:, NPAIR + p:NPAIR + p + 1] for p in range(NPAIR)]
            bias_sin = auxt[:, 2 * NPAIR:2 * NPAIR + 1]   # -pi
            bias_cos = auxt[:, 2 * NPAIR + 1:2 * NPAIR + 2]  # -pi/2

            # A tiny Sin as the very first ACT instruction makes walrus load
            # the trig table set up front (Copy lives in every set), so the
            # whole program needs just one mid-stream switch (to Exp).
            warm = csb.tile([128, 1], f32, name="warm")
            nc.scalar.activation(warm, auxt[:, 0:1],
                                 mybir.ActivationFunctionType.Sin,
                                 bias=0.0, scale=0.0)

            # ---- projections (dup rows: h in 0:64 == 64:128) ----
            # Queries first: the Q-side features gate every scores matmul,
            # so they are produced before the (much larger) K-side work.
            QSHIFT = float(1 << (FRAC_BITS - 2))
            SINSC = float(2 * np.pi / QSC)
            MASK = (1 << FRAC_BITS) - 1

            # ---- K-side projections staged to SBUF via ACT copies ----
            kps_sb = bsb.tile([128, KW], f32, name="kps_sb")
            for c in range(NCH):
                kps = kps_pool.tile([128, 512], f32, name="kps")
                for t in range(2):
                    nc.tensor.matmul(
                        kps,
                        wqkt[:, t * 128:(t + 1) * 128],
                        kT_sb[:, t * KW + c * 512: t * KW + (c + 1) * 512],
                        start=(t == 0), stop=(t == 1))
                if c % 2 == 0:
                    nc.scalar.activation(
                        kps_sb[:, c * 512:(c + 1) * 512], kps,
                        mybir.ActivationFunctionType.Copy, bias=0.0, scale=1.0)
                else:
                    nc.vector.tensor_copy(kps_sb[:, c * 512:(c + 1) * 512], kps)

            # Q-side U features are host-staged (uq_st): tiny fraction of
            # the FLOPs but disproportionate device instruction overhead.
            uq = [(uq_sb[:, (2 * p) * QW:(2 * p + 1) * QW],
                   uq_sb[:, (2 * p + 1) * QW:(2 * p + 2) * QW])
                  for p in range(NPAIR)]

            # K-side features: fk[p][0] = -sin(K), fk[p][1] = -cos(K)
            fk = []
            for p in range(NPAIR):
                kpair = []
                for quarter in (0.0, QSHIFT):
                    ik = wsb.tile([128, KW], i32, name="ik", tag="ik", bufs=(3 if nslot <= 5 else 1))
                    nc.vector.tensor_scalar(
                        ik, kps_sb, th_ap[p], quarter or None,
                        mybir.AluOpType.mult,
                        mybir.AluOpType.add if quarter else mybir.AluOpType.bypass)
                    mk = wsb.tile([128, KW], i32, name="mk", tag="mk", bufs=3)
                    nc.vector.tensor_scalar(mk, ik, MASK, None,
                                            mybir.AluOpType.bitwise_and)
                    fkt = bsb.tile([128, KW], f16, name=f"fk{p}_{int(bool(quarter))}")
                    nc.scalar.activation(fkt, mk,
                                         mybir.ActivationFunctionType.Sin,
                                         bias=bias_sin, scale=SINSC)
                    kpair.append(fkt)
                fk.append(kpair)

            # ---- scores^T, exp, PV ----
            # Two slots share one score tile (regions stay inside banks);
            # halves the exp instruction count.
            out_sb = csb.tile([128, nslot * 132], f32, name="out_sb")
            nj = 2 * NPAIR
            for s0 in range(0, nslot, 2):
                npair_s = min(2, nslot - s0)
                w = npair_s * 512
                sct = sc_pool.tile([128, 1024], f32, name="sct")
                for ds in range(npair_s):
                    s = s0 + ds
                    for u in range(UPS):
                        col = (s * UPS + u) * 128
                        oc = ds * 512 + u * 128
                        for jp in range(NPAIR):
                            for ab in range(2):
                                j = jp * 2 + ab
                                nc.tensor.matmul(
                                    sct[:, oc:oc + 128],
                                    fk[jp][ab][:, col:col + 128],
                                    uq[jp][1 - ab][:, s * 128:(s + 1) * 128],
                                    start=(j == 0), stop=(j == nj - 1))
                pt = wsb.tile([128, 1024], f16, name="pt", tag="pt")
                nc.scalar.activation(pt[:, 0:w], sct[:, 0:w],
                                     mybir.ActivationFunctionType.Exp,
                                     bias=0.0, scale=1.0)
                for ds in range(npair_s):
                    s = s0 + ds
                    pv = pv_pool.tile([128, 132], f32, name="pv", tag="pv")
                    for u in range(UPS):
                        ucol = (s * UPS + u) * 132
                        nc.tensor.matmul(
                            pv[:, 0:129],
                            pt[:, ds * 512 + u * 128: ds * 512 + (u + 1) * 128],
                            vals_sb[:, ucol:ucol + 129],
                            start=(u == 0), stop=(u == UPS - 1))
                    nc.vector.tensor_copy(out_sb[:, s * 132:(s + 1) * 132], pv)

            nc.sync.dma_start(out=out_part[:, :], in_=out_sb)

    nc.compile()
    return nc


def _get_program(nslot):
    with _prog_lock:
        if nslot not in _prog_cache:
            _prog_cache[nslot] = _build_program(nslot)
        return _prog_cache[nslot]


def kernel(queries, keys, values, valid_lens, Wq, Wk, wv):
    queries = np.asarray(queries, np.float32)
    keys = np.asarray(keys, np.float32)
    values = np.asarray(values, np.float32)
    valid_lens = np.asarray(valid_lens, np.int32)
    Wq = np.asarray(Wq, np.float32)
    Wk = np.asarray(Wk, np.float32)
    wv = np.asarray(wv, np.float32)

    # ---- host: per-h ranges -> table rows ----
    qp = (queries.reshape(-1, DQ) @ Wq).reshape(B, NQ, DH)
    qmax = np.abs(qp).max(axis=(0, 1))
    kmax = np.zeros(DH, np.float64)
    kp_valid_max = np.zeros(DH)
    for b in range(B):
        L = int(valid_lens[b])
        kp = keys[b, :L] @ Wk
        kp_valid_max = np.maximum(kp_valid_max, np.abs(kp).max(axis=0))
    Sh = (qmax + kp_valid_max) * 1.0005
    THm = np.zeros((DH, M), np.float64)
    Cm = np.zeros((DH, M), np.float64)
    Sg_h = np.zeros(DH)
    for h in range(DH):
        idx = min(int(np.searchsorted(SGRID, Sh[h])), len(SGRID) - 1)
        Sg = float(SGRID[idx])
        th, cc = THETA_TABLE[round(Sg, 2)]
        THm[h] = th
        Cm[h] = cc
        Sg_h[h] = Sg
    bh = (1.0 / (2 * np.pi * Sg_h)).astype(np.float32)

    # ---- host: bin-pack (batch, key-tile) groups onto cores ----
    groups = []  # (batch, tile_start, n_tiles<=UPS)
    for b in range(B):
        T = -(-int(valid_lens[b]) // TILE)
        for g0 in range(0, T, UPS):
            groups.append((b, g0, min(UPS, T - g0)))
    groups.sort(key=lambda g: -g[2])
    core_slots = [[] for _ in range(N_CORES)]
    load = [0] * N_CORES
    for g in groups:
        c = int(np.argmin(load))
        core_slots[c].append(g)
        load[c] += g[2]
    nslot = max(1, max(len(s) for s in core_slots))
    units = nslot * UPS
    KW = units * TILE
    QW = nslot * TILE

    # ---- host: staging ----
    Wk_b = (Wk * bh[None, :]).astype(np.float32)
    wqk = np.zeros((128, 256), np.float16)
    for t in range(2):
        blk = Wk_b[t * 128:(t + 1) * 128]          # [128, 64]
        wqk[:, t * 128:t * 128 + 64] = blk
        wqk[:, t * 128 + 64:(t + 1) * 128] = blk   # dup rows of projection output

    auxw = NPAIR + NPAIR + 2
    auxv = np.zeros((128, auxw), np.float32)
    for p in range(NPAIR):
        auxv[0:64, p] = THm[:, 2 * p] * QSC
        auxv[64:128, p] = THm[:, 2 * p + 1] * QSC
        auxv[0:64, NPAIR + p] = Cm[:, 2 * p] * wv
        auxv[64:128, NPAIR + p] = Cm[:, 2 * p + 1] * wv
    auxv[:, 2 * NPAIR] = -np.pi
    auxv[:, 2 * NPAIR + 1] = -np.pi / 2

    OMqh = (THm / Sg_h[:, None])          # [DH, M]: omega = theta / S_g
    CWm = [(Cm[:, m] * wv) for m in range(M)]  # [DH] each
    in_maps = []
    slot_meta = []  # per core: list of batch ids (or -1)
    for c in range(N_CORES):
        kT = np.zeros((2, 128, KW), np.float16)
        uqa = np.zeros((128, 2 * NPAIR * QW), np.float16)
        vals = np.zeros((128, units * 132), np.float16)
        meta = []
        for s, (b, g0, ntiles) in enumerate(core_slots[c]):
            meta.append(b)
            L = int(valid_lens[b])
            ang = qp[b][:, :, None] * OMqh[None, :, :]   # [NQ, DH, M]
            for p in range(NPAIR):
                for half in range(2):
                    m = 2 * p + half
                    rows = slice(half * 64, (half + 1) * 64)
                    a = ang[:, :, m].T                    # [DH, NQ]
                    cw = CWm[m][:, None]                  # [DH, 1]
                    uqa[rows, (2 * p) * QW + s * 128:(2 * p) * QW + (s + 1) * 128] = \
                        (-np.sin(a) * cw).astype(np.float16)
                    uqa[rows, (2 * p + 1) * QW + s * 128:(2 * p + 1) * QW + (s + 1) * 128] = \
                        (-np.cos(a) * cw).astype(np.float16)
            for u in range(ntiles):
                k0 = (g0 + u) * TILE
                k1 = min(k0 + TILE, L)
                n = k1 - k0
                col = (s * UPS + u) * TILE
                kblk = keys[b, k0:k1].T  # [DK, n]
                for t in range(2):
                    kT[t, :, col:col + n] = kblk[t * 128:(t + 1) * 128]
                ucol = (s * UPS + u) * 132
                vals[:n, ucol:ucol + DV] = values[b, k0:k1]
                vals[:n, ucol + DV] = 1.0
        while len(meta) < nslot:
            meta.append(-1)
        slot_meta.append(meta)
        in_maps.append({"kT_st": kT, "uq_st": uqa, "vals_st": vals,
                        "wqk_st": wqk, "aux": auxv})

    # ---- run on 8 cores ----
    nc = _get_program(nslot)
    trace = bool(int(os.environ.get("ADDATTN_TRACE", "0")))
    res = run_bass_kernel_spmd(nc, in_maps, core_ids=list(range(N_CORES)),
                               trace=trace)
    if trace:
        kernel.last_results = res

    # ---- host: unshard (sum partials, normalize) ----
    acc = np.zeros((B, NQ, DV + 1), np.float64)
    for c in range(N_CORES):
        part = res.results[c]["out_part"]  # [128, nslot*132]
        for s, b in enumerate(slot_meta[c]):
            if b < 0:
                continue
            acc[b] += part[:, s * 132: s * 132 + DV + 1].astype(np.float64)
    out = (acc[:, :, :DV] / acc[:, :, DV:DV + 1]).astype(np.float32)
    return out


# revision 29
# speedup vs baseline: 1.0391x; 1.0391x over previous
"""Additive (Bahdanau) attention on 8 Trainium2 NeuronCores.

Strategy
--------
scores[b,q,k] = sum_h wv[h] * tanh(qp[b,q,h] + kp[b,k,h]) is evaluated via a
separable sin-basis expansion  tanh(S*x) ~ sum_m c_m sin(theta_m * x), so that
sin(w*(qp+kp)) = sin(w qp)cos(w kp) + cos(w qp)sin(w kp) turns the O(nq*nk*dh)
tanh work into PE matmuls over 2*M*dh fused feature columns.  Feature angles
are range-reduced to the hardware Sin table domain [-pi, pi] with an int32
fixed-point fraction trick on the vector engine.  Masked softmax is exact:
only valid key tiles are ever computed (work is bin-packed over cores at
128-key-tile granularity), the denominator comes from an extra all-ones
column in the PV matmul, and per-(core,slot) partial numerators/denominators
are summed and normalized on the host.

Each core runs the same program; per-core work differences are expressed
purely through host-staged input tensors.
"""

import os
import threading

import numpy as np

import concourse.bacc as bacc
import concourse.mybir as mybir
from concourse.tile import TileContext
from concourse.bass_utils import run_bass_kernel_spmd

# Problem constants (nn_AdditiveAttention_48859547959476).
B, NQ, NK, DQ, DK, DV, DH = 16, 128, 2048, 256, 256, 128, 64
N_CORES = 8
M = 8                  # sin modes per h
NPAIR = M // 2         # two modes packed per 128-partition instruction
FRAC_BITS = 18
QSC = float(2 ** FRAC_BITS)
TILE = 128             # key-tile granularity
UPS = 4                # units (key tiles) per slot; one slot = one batch segment

# Fitted sin-basis tables: tanh(S*x) ~ sum_m c_m * sin(theta_m * x), x in [-1,1].
# Data-independent constants (depend only on the interval half-width S grid).
THETA_TABLE = {
    6.5: ([3.17193937, 9.60585883, 2.18738104, 3.05597649, 14.9417154, 20.4502654, 26.1547075, 31.9750537],
           [36.2232438, 0.161350491, 6.87426557, -41.2400577, 0.0450386597, 0.0123209328, 0.0032060403, 0.000771167043]),
    7.0: ([6.72542412, 11.4488434, 0.860825183, 2.30642042, 16.467999, 21.7569349, 27.2808087, 32.9082271],
           [0.304060007, 0.107251691, 0.22853436, 1.09560141, 0.0366555054, 0.0117308915, 0.00351331224, 0.000959944543]),
    7.5: ([7.25768278, 12.6007308, 0.94197816, 0.389083886, 18.0598494, 23.6786715, 29.4473074, 35.2574239],
           [0.327218102, 0.103098701, 18.1001284, -36.5762453, 0.0336387636, 0.0106648096, 0.00325694801, 0.000927483766]),
    8.0: ([7.62827148, 12.9275591, 2.38525745, 0.0910723624, 18.386375, 24.0077522, 29.7720771, 35.5527541],
           [0.307919105, 0.106834021, 1.40006375, -2.92600933, 0.03750095, 0.0127778322, 0.00420384838, 0.00129178369]),
    8.5: ([8.57452428, 13.6394417, 3.96189912, 0.0316319922, 18.9814717, 24.5275185, 30.2180244, 35.8879972],
           [0.245844862, 0.0994266193, 0.631129752, 37.8639255, 0.0384797017, 0.0142488136, 0.00507491304, 0.00166977822]),
    9.0: ([10.5059062, 15.9257514, 5.22232869, 0.281150478, 21.4885597, 27.1845335, 32.985403, 38.7556262],
           [0.194960676, 0.0760384553, 0.556324013, 5.93191099, 0.0294364649, 0.0111261319, 0.00409146713, 0.00140742744]),
    9.5: ([10.6455037, 16.092124, 5.34363126, 0.631205216, 21.6748922, 27.381036, 33.1809849, 38.9279582],
           [0.200629478, 0.0815646864, 0.547398048, 2.77132144, 0.0330422178, 0.0131083071, 0.00507118009, 0.00183458507]),
    10.0: ([10.7886067, 16.2530977, 5.48579349, 0.913103266, 21.850584, 27.5632467, 33.3571549, 39.0815875],
           [0.204992033, 0.0866742822, 0.533127269, 2.02948997, 0.036611695, 0.0151849651, 0.00615193949, 0.00232647664]),
    10.5: ([10.9868662, 16.4488031, 5.73180365, 1.30190719, 22.0509821, 27.7667963, 33.5572858, 39.2669982],
           [0.205719804, 0.0906935634, 0.500097882, 1.5731762, 0.0398856941, 0.0172431936, 0.00729298612, 0.0028792393]),
}
SGRID = np.array(sorted(THETA_TABLE.keys()))

_prog_cache = {}
_prog_lock = threading.Lock()


def _build_program(nslot):
    """One Bass/Tile program shared by all 8 cores.

    Inputs (per core, staged by the host):
      kT_st   [2, 128, U*128] f16 : transposed keys (dk-tile major), zero-padded
      uq_st   [128, 8*S*128]  f16 : host-computed Q-side U features
      vals_st [128, U*132]    f16 : per unit: 128 value cols + mask col + 3 pad
      wqk_st  [128, 256]      f16 : WkWk dup'd (dk-tile major), 1/(2pi*Sg) folded
      aux     [128, A]        f32 : theta*2^18 pair vecs, U coefs, biases
    Output:
      out_part [128, S*132] f32 : per slot: 128 PV cols + denominator col + 3 pad
    """
    units = nslot * UPS
    KW = units * TILE          # staged key columns
    QW = nslot * TILE          # staged query columns
    AW = NPAIR + NPAIR + 2     # aux cols: theta pair vecs, ucoef pair vecs, biases
    f32, f16, i32 = mybir.dt.float32, mybir.dt.float16, mybir.dt.int32

    nc = bacc.Bacc("TRN2", target_bir_lowering=False, debug=False,
                   num_devices=N_CORES)
    kT_st = nc.dram_tensor("kT_st", [2, 128, KW], f16, kind="ExternalInput").ap()
    uq_st = nc.dram_tensor("uq_st", [128, 2 * NPAIR * QW], f16, kind="ExternalInput").ap()
    vals_st = nc.dram_tensor("vals_st", [128, units * 132], f16, kind="ExternalInput").ap()
    wqk_st = nc.dram_tensor("wqk_st", [128, 256], f16, kind="ExternalInput").ap()
    aux = nc.dram_tensor("aux", [128, AW], f32, kind="ExternalInput").ap()
    out_part = nc.dram_tensor("out_part", [128, nslot * 132], f32, kind="ExternalOutput").ap()

    NCH = KW // 512            # 512-col key-projection chunks
    FW = KW + QW               # fused feature width (keys then queries)

    with TileContext(nc) as tc:
        with (
            tc.tile_pool(name="const_sb", bufs=1) as csb,
            tc.tile_pool(name="big_sb", bufs=1) as bsb,
            tc.tile_pool(name="work_sb", bufs=2) as wsb,
            tc.tile_pool(name="kps_ps", bufs=3, space="PSUM") as kps_pool,
            tc.tile_pool(name="sc_ps", bufs=2, space="PSUM") as sc_pool,
            tc.tile_pool(name="pv_ps", bufs=1, space="PSUM") as pv_pool,
        ):
            auxt = csb.tile([128, AW], f32, name="auxt")
            nc.sync.dma_start(out=auxt, in_=aux[:, :])
            wqkt = csb.tile([128, 256], f16, name="wqkt")
            nc.sync.dma_start(out=wqkt, in_=wqk_st[:, :])
            kT_sb = bsb.tile([128, 2 * KW], f16, name="kT_sb")
            for c in range(NCH):
                for t in range(2):
                    nc.sync.dma_start(
                        out=kT_sb[:, t * KW + c * 512: t * KW + (c + 1) * 512],
                        in_=kT_st[t, :, c * 512:(c + 1) * 512])
            uq_sb = csb.tile([128, 2 * NPAIR * QW], f16, name="uq_sb")
            nc.sync.dma_start(out=uq_sb, in_=uq_st[:, :])
            vals_sb = bsb.tile([128, units * 132], f16, name="vals_sb")
            nc.sync.dma_start(out=vals_sb, in_=vals_st[:, :])

            th_ap = [auxt[:, p:p + 1] for p in range(NPAIR)]
            uc_ap = [auxt[:, NPAIR + p:NPAIR + p + 1] for p in range(NPAIR)]
            bias_sin = auxt[:, 2 * NPAIR:2 * NPAIR + 1]   # -pi
            bias_cos = auxt[:, 2 * NPAIR + 1:2 * NPAIR + 2]  # -pi/2

            # A tiny Sin as the very first ACT instruction makes walrus load
            # the trig table set up front (Copy lives in every set), so the
            # whole program needs just one mid-stream switch (to Exp).
            warm = csb.tile([128, 1], f32, name="warm")
            nc.scalar.activation(warm, auxt[:, 0:1],
                                 mybir.ActivationFunctionType.Sin,
                                 bias=0.0, scale=0.0)

            # ---- projections (dup rows: h in 0:64 == 64:128) ----
            # Queries first: the Q-side features gate every scores matmul,
            # so they are produced before the (much larger) K-side work.
            QSHIFT = float(1 << (FRAC_BITS - 2))
            SINSC = float(2 * np.pi / QSC)
            MASK = (1 << FRAC_BITS) - 1

            # ---- K-side projections staged to SBUF via ACT copies ----
            kps_sb = bsb.tile([128, KW], f32, name="kps_sb")
            for c in range(NCH):
                kps = kps_pool.tile([128, 512], f32, name="kps")
                for t in range(2):
                    nc.tensor.matmul(
                        kps,
                        wqkt[:, t * 128:(t + 1) * 128],
                        kT_sb[:, t * KW + c * 512: t * KW + (c + 1) * 512],
                        start=(t == 0), stop=(t == 1))
                if c % 2 == 0:
                    nc.scalar.activation(
                        kps_sb[:, c * 512:(c + 1) * 512], kps,
                        mybir.ActivationFunctionType.Copy, bias=0.0, scale=1.0)
                else:
                    nc.vector.tensor_copy(kps_sb[:, c * 512:(c + 1) * 512], kps)

            # Q-side U features are host-staged (uq_st): tiny fraction of
            # the FLOPs but disproportionate device instruction overhead.
            uq = [(uq_sb[:, (2 * p) * QW:(2 * p + 1) * QW],
                   uq_sb[:, (2 * p + 1) * QW:(2 * p + 2) * QW])
                  for p in range(NPAIR)]

            # K-side features: fk[p][0] = -sin(K), fk[p][1] = -cos(K).
            # Pair 0's chain is split at a chunk boundary so its first half
            # (and with it the PE score stream) starts before the last key
            # projection chunk lands.
            HSPLIT = min(3 * 512, KW)
            fk = []
            for p in range(NPAIR):
                kpair = []
                for quarter in (0.0, QSHIFT):
                    ik = wsb.tile([128, KW], i32, name="ik", tag="ik", bufs=(3 if nslot <= 5 else 1))
                    mk = wsb.tile([128, KW], i32, name="mk", tag="mk", bufs=3)
                    fkt = bsb.tile([128, KW], f16, name=f"fk{p}_{int(bool(quarter))}")
                    parts = ((0, HSPLIT), (HSPLIT, KW)) if (p == 0 and HSPLIT < KW) \
                        else ((0, KW),)
                    for lo, hi in parts:
                        nc.vector.tensor_scalar(
                            ik[:, lo:hi], kps_sb[:, lo:hi], th_ap[p],
                            quarter or None,
                            mybir.AluOpType.mult,
                            mybir.AluOpType.add if quarter else mybir.AluOpType.bypass)
                        nc.vector.tensor_scalar(mk[:, lo:hi], ik[:, lo:hi],
                                                MASK, None,
                                                mybir.AluOpType.bitwise_and)
                        nc.scalar.activation(fkt[:, lo:hi], mk[:, lo:hi],
                                             mybir.ActivationFunctionType.Sin,
                                             bias=bias_sin, scale=SINSC)
                    kpair.append(fkt)
                fk.append(kpair)

            # ---- scores^T, exp, PV ----
            # Two slots share one score tile (regions stay inside banks);
            # halves the exp instruction count.
            out_sb = csb.tile([128, nslot * 132], f32, name="out_sb")
            nj = 2 * NPAIR
            for s0 in range(0, nslot, 2):
                npair_s = min(2, nslot - s0)
                w = npair_s * 512
                sct = sc_pool.tile([128, 1024], f32, name="sct")
                for ds in range(npair_s):
                    s = s0 + ds
                    for u in range(UPS):
                        col = (s * UPS + u) * 128
                        oc = ds * 512 + u * 128
                        for jp in range(NPAIR):
                            for ab in range(2):
                                j = jp * 2 + ab
                                nc.tensor.matmul(
                                    sct[:, oc:oc + 128],
                                    fk[jp][ab][:, col:col + 128],
                                    uq[jp][1 - ab][:, s * 128:(s + 1) * 128],
                                    start=(j == 0), stop=(j == nj - 1))
                pt = wsb.tile([128, 1024], f16, name="pt", tag="pt")
                nc.scalar.activation(pt[:, 0:w], sct[:, 0:w],
                                     mybir.ActivationFunctionType.Exp,
                                     bias=0.0, scale=1.0)
                for ds in range(npair_s):
                    s = s0 + ds
                    pv = pv_pool.tile([128, 132], f32, name="pv", tag="pv")
                    for u in range(UPS):
                        ucol = (s * UPS + u) * 132
                        nc.tensor.matmul(
                            pv[:, 0:129],
                            pt[:, ds * 512 + u * 128: ds * 512 + (u + 1) * 128],
                            vals_sb[:, ucol:ucol + 129],
                            start=(u == 0), stop=(u == UPS - 1))
                    nc.vector.tensor_copy(out_sb[:, s * 132:(s + 1) * 132], pv)

            nc.sync.dma_start(out=out_part[:, :], in_=out_sb)

    nc.compile()
    return nc


def _get_program(nslot):
    with _prog_lock:
        if nslot not in _prog_cache:
            _prog_cache[nslot] = _build_program(nslot)
        return _prog_cache[nslot]


def kernel(queries, keys, values, valid_lens, Wq, Wk, wv):
    queries = np.asarray(queries, np.float32)
    keys = np.asarray(keys, np.float32)
    values = np.asarray(values, np.float32)
    valid_lens = np.asarray(valid_lens, np.int32)
    Wq = np.asarray(Wq, np.float32)
    Wk = np.asarray(Wk, np.float32)
    wv = np.asarray(wv, np.float32)

    # ---- host: per-h ranges -> table rows ----
    qp = (queries.reshape(-1, DQ) @ Wq).reshape(B, NQ, DH)
    qmax = np.abs(qp).max(axis=(0, 1))
    kmax = np.zeros(DH, np.float64)
    kp_valid_max = np.zeros(DH)
    for b in range(B):
        L = int(valid_lens[b])
        kp = keys[b, :L] @ Wk
        kp_valid_max = np.maximum(kp_valid_max, np.abs(kp).max(axis=0))
    Sh = (qmax + kp_valid_max) * 1.0005
    THm = np.zeros((DH, M), np.float64)
    Cm = np.zeros((DH, M), np.float64)
    Sg_h = np.zeros(DH)
    for h in range(DH):
        idx = min(int(np.searchsorted(SGRID, Sh[h])), len(SGRID) - 1)
        Sg = float(SGRID[idx])
        th, cc = THETA_TABLE[round(Sg, 2)]
        THm[h] = th
        Cm[h] = cc
        Sg_h[h] = Sg
    bh = (1.0 / (2 * np.pi * Sg_h)).astype(np.float32)

    # ---- host: bin-pack (batch, key-tile) groups onto cores ----
    groups = []  # (batch, tile_start, n_tiles<=UPS)
    for b in range(B):
        T = -(-int(valid_lens[b]) // TILE)
        for g0 in range(0, T, UPS):
            groups.append((b, g0, min(UPS, T - g0)))
    groups.sort(key=lambda g: -g[2])
    core_slots = [[] for _ in range(N_CORES)]
    load = [0] * N_CORES
    for g in groups:
        c = int(np.argmin(load))
        core_slots[c].append(g)
        load[c] += g[2]
    nslot = max(1, max(len(s) for s in core_slots))
    units = nslot * UPS
    KW = units * TILE
    QW = nslot * TILE

    # ---- host: staging ----
    Wk_b = (Wk * bh[None, :]).astype(np.float32)
    wqk = np.zeros((128, 256), np.float16)
    for t in range(2):
        blk = Wk_b[t * 128:(t + 1) * 128]          # [128, 64]
        wqk[:, t * 128:t * 128 + 64] = blk
        wqk[:, t * 128 + 64:(t + 1) * 128] = blk   # dup rows of projection output

    auxw = NPAIR + NPAIR + 2
    auxv = np.zeros((128, auxw), np.float32)
    for p in range(NPAIR):
        auxv[0:64, p] = THm[:, 2 * p] * QSC
        auxv[64:128, p] = THm[:, 2 * p + 1] * QSC
        auxv[0:64, NPAIR + p] = Cm[:, 2 * p] * wv
        auxv[64:128, NPAIR + p] = Cm[:, 2 * p + 1] * wv
    auxv[:, 2 * NPAIR] = -np.pi
    auxv[:, 2 * NPAIR + 1] = -np.pi / 2

    OMqh = (THm / Sg_h[:, None])          # [DH, M]: omega = theta / S_g
    CWm = [(Cm[:, m] * wv) for m in range(M)]  # [DH] each
    in_maps = []
    slot_meta = []  # per core: list of batch ids (or -1)
    for c in range(N_CORES):
        kT = np.zeros((2, 128, KW), np.float16)
        uqa = np.zeros((128, 2 * NPAIR * QW), np.float16)
        vals = np.zeros((128, units * 132), np.float16)
        meta = []
        for s, (b, g0, ntiles) in enumerate(core_slots[c]):
            meta.append(b)
            L = int(valid_lens[b])
            ang = qp[b][:, :, None] * OMqh[None, :, :]   # [NQ, DH, M]
            for p in range(NPAIR):
                for half in range(2):
                    m = 2 * p + half
                    rows = slice(half * 64, (half + 1) * 64)
                    a = ang[:, :, m].T                    # [DH, NQ]
                    cw = CWm[m][:, None]                  # [DH, 1]
                    uqa[rows, (2 * p) * QW + s * 128:(2 * p) * QW + (s + 1) * 128] = \
                        (-np.sin(a) * cw).astype(np.float16)
                    uqa[rows, (2 * p + 1) * QW + s * 128:(2 * p + 1) * QW + (s + 1) * 128] = \
                        (-np.cos(a) * cw).astype(np.float16)
            for u in range(ntiles):
                k0 = (g0 + u) * TILE
                k1 = min(k0 + TILE, L)
                n = k1 - k0
                col = (s * UPS + u) * TILE
                kblk = keys[b, k0:k1].T  # [DK, n]
                for t in range(2):
                    kT[t, :, col:col + n] = kblk[t * 128:(t + 1) * 128]
                ucol = (s * UPS + u) * 132
                vals[:n, ucol:ucol + DV] = values[b, k0:k1]
                vals[:n, ucol + DV] = 1.0
        while len(meta) < nslot:
            meta.append(-1)
        slot_meta.append(meta)
        in_maps.append({"kT_st": kT, "uq_st": uqa, "vals_st": vals,
                        "wqk_st": wqk, "aux": auxv})

    # ---- run on 8 cores ----
    nc = _get_program(nslot)
    trace = bool(int(os.environ.get("ADDATTN_TRACE", "0")))
    res = run_bass_kernel_spmd(nc, in_maps, core_ids=list(range(N_CORES)),
                               trace=trace)
    if trace:
        kernel.last_results = res

    # ---- host: unshard (sum partials, normalize) ----
    acc = np.zeros((B, NQ, DV + 1), np.float64)
    for c in range(N_CORES):
        part = res.results[c]["out_part"]  # [128, nslot*132]
        for s, b in enumerate(slot_meta[c]):
            if b < 0:
                continue
            acc[b] += part[:, s * 132: s * 132 + DV + 1].astype(np.float64)
    out = (acc[:, :, :DV] / acc[:, :, DV:DV + 1]).astype(np.float32)
    return out
